# revision 1
# baseline (speedup 1.0000x reference)
"""NT-Xent loss kernel, v2.

vs v1: batched multi-dim DMAs (SP seq issue cost was 126us in v1),
bf16 sim-path matmuls (1 cycle/row vs fp32 4), normalize
multiplies moved to GpSimd, pipelined phase C chunk-pairs, ACT ln/exp
table warmup under the DMA head.
"""

import numpy as np
from contextlib import ExitStack

N = 4096
D = 256
C = 100
B = 2048
N_CORES = 8
RPC = 512
E_CONST = float(np.e)

TS_ON_GPSIMD = True

_PROG = None


def _build_program():
    import concourse.bass as bass
    import concourse.tile as tile
    from concourse import bacc, mybir, masks

    f32 = mybir.dt.float32
    f32r = mybir.dt.float32r
    bf16 = mybir.dt.bfloat16
    MULT = mybir.AluOpType.mult
    ADD = mybir.AluOpType.add
    EXP = mybir.ActivationFunctionType.Exp
    LN = mybir.ActivationFunctionType.Ln
    AX = mybir.AxisListType.X

    nc = bacc.Bacc(
        "TRN2",
        target_bir_lowering=False,
        debug=False,
        enable_asserts=False,
        num_devices=N_CORES,
    )

    z_full = nc.dram_tensor("z_full", [N, D], f32, kind="ExternalInput").ap()
    zq = nc.dram_tensor("zq", [RPC, D], f32, kind="ExternalInput").ap()
    dist = nc.dram_tensor("dist", [B, C], f32, kind="ExternalInput").ap()
    distq = nc.dram_tensor("distq", [RPC, C], f32, kind="ExternalInput").ap()
    out = nc.dram_tensor("out", [128, 9], f32, kind="ExternalOutput").ap()

    with tile.TileContext(nc) as tc, ExitStack() as ctx:
        per = ctx.enter_context(tc.tile_pool(name="persist", bufs=1))

        ident = per.tile([128, 128], f32)
        masks.make_identity(nc, ident[:])

        # ln/exp table warmup (kept live via out[:, 8])
        warm = per.tile([128, 1], f32)
        nc.vector.memset(warm[:], 1.0)
        nc.scalar.activation(warm[:], warm[:], LN)
        nc.scalar.activation(warm[:], warm[:], EXP, scale=-0.5)

        zn = per.tile([128, 32 * 256], f32)
        zqn = per.tile([128, 4 * 256], f32)
        zT = per.tile([128, 2 * 4096], bf16)
        zqT = per.tile([128, 2 * 512], f32)
        zqT_bf = per.tile([128, 2 * 512], bf16)
        dist_sb = per.tile([128, 16 * C], f32)
        distq_sb = per.tile([128, 4 * C], f32)
        norm2 = per.tile([128, 36], f32)
        lnn = per.tile([128, 36], f32)
        rn = per.tile([128, 36], f32)
        junk = per.tile([128, 256], f32)
        wt_sb = per.tile([128, 256], f32)
        w_sb = per.tile([128, 2 * C], f32)
        q_sb = per.tile([128, C], f32)
        zsum = per.tile([128, 16 * 256], f32)
        S_parts = per.tile([128, 16], f32)
        S_out = per.tile([128, 4], f32)
        P_out = per.tile([128, 4], f32)
        exp_scr = per.tile([128, 1024], f32)

        # ---------------- batched loads ----------------
        # zq: one DMA [512, 256] -> [128, 4, 256]
        nc.sync.dma_start(
            out=zqn[:].rearrange("p (t j) -> p t j", j=256),
            in_=zq.rearrange("(t p) j -> p t j", p=128),
        )
        # z: 4 DMAs of 8 row-tiles each (pipelines with norms/transposes)
        for g in range(4):
            nc.sync.dma_start(
                out=zn[:, g * 2048:(g + 1) * 2048].rearrange("p (t j) -> p t j", j=256),
                in_=z_full[g * 1024:(g + 1) * 1024, :].rearrange("(t p) j -> p t j", p=128),
            )
        nc.sync.dma_start(
            out=dist_sb[:].rearrange("p (t c) -> p t c", c=C),
            in_=dist.rearrange("(t p) c -> p t c", p=128),
        )
        nc.sync.dma_start(
            out=distq_sb[:].rearrange("p (t c) -> p t c", c=C),
            in_=distq.rearrange("(t p) c -> p t c", p=128),
        )

        def nat_tile(t):
            # norm2 column t: 0-3 -> zq tiles, 4-35 -> z tiles t-4
            return zqn[:, t * 256:(t + 1) * 256] if t < 4 else \
                zn[:, (t - 4) * 256:(t - 3) * 256]

        # ---------------- norms: one mul+reduce per 4-8 tile group ----------------
        with tc.tile_pool(name="sqp", bufs=2) as sqp:
            sq = sqp.tile([128, 1024], f32, tag="sq")
            nc.vector.tensor_mul(sq[:], zqn[:], zqn[:])
            nc.vector.tensor_reduce(
                out=norm2[:, 0:4],
                in_=sq[:].rearrange("p (t j) -> p t j", j=256), axis=AX, op=ADD,
            )
            for g in range(4):
                sq = sqp.tile([128, 2048], f32, tag="sqz")
                src = zn[:, g * 2048:(g + 1) * 2048]
                nc.vector.tensor_mul(sq[:], src, src)
                nc.vector.tensor_reduce(
                    out=norm2[:, 4 + g * 8:4 + (g + 1) * 8],
                    in_=sq[:].rearrange("p (t j) -> p t j", j=256), axis=AX, op=ADD,
                )

        for g in range(3):
            sl = slice(g * 12, (g + 1) * 12)
            nc.scalar.activation(lnn[:, sl], norm2[:, sl], LN)
            nc.scalar.activation(rn[:, sl], lnn[:, sl], EXP, scale=-0.5)

        for t in range(36):
            src = nat_tile(t)
            eng = nc.gpsimd if TS_ON_GPSIMD else nc.vector
            eng.tensor_scalar(out=src, in0=src, scalar1=rn[:, t:t + 1],
                              scalar2=None, op0=MULT)

        # phase C pool first (6 banks) + transpose pool (2 banks) concurrently;
        # nominator pool reuses the transpose banks after they release.
        with tc.tile_pool(name="psum_c", bufs=3, space="PSUM") as pc:
            with tc.tile_pool(name="psum_tr", bufs=2, space="PSUM") as ptr:
                # transposes: zq first, then z groups. Early-group PSUM->SBUF
                # copies go to ScalarE (idle during the DMA/norm head, and its
                # PSUM port is fast); DVE is saturated by norms then.
                for d in range(2):
                    pt = ptr.tile([128, 512], f32, tag="tr")
                    for k in range(4):
                        nc.tensor.transpose(
                            pt[:, k * 128:(k + 1) * 128],
                            zqn[:, k * 256 + d * 128: k * 256 + d * 128 + 128],
                            ident[:],
                        )
                    nc.scalar.copy(out=zqT[:, d * 512:(d + 1) * 512], in_=pt[:])
                    nc.vector.tensor_copy(out=zqT_bf[:, d * 512:(d + 1) * 512], in_=pt[:])
                for g in range(8):
                    for d in range(2):
                        pt = ptr.tile([128, 512], f32, tag="tr")
                        for k in range(4):
                            t = g * 4 + k
                            nc.tensor.transpose(
                                pt[:, k * 128:(k + 1) * 128],
                                zn[:, t * 256 + d * 128: t * 256 + d * 128 + 128],
                                ident[:],
                            )
                        dst = zT[:, d * 4096 + g * 512: d * 4096 + (g + 1) * 512]
                        if g < 4:
                            nc.scalar.copy(out=dst, in_=pt[:])
                        else:
                            nc.vector.tensor_copy(out=dst, in_=pt[:])

            # dist rows repeat (rows t and t+16 share labels), so fold the two
            # z chunks sharing each dist lhsT: W^T = sum_t distT_t @ (zn_t + zn_t+16)
            # -> 16 fp32 matmuls instead of 32. DVE is idle in this window.
            for t in range(16):
                nc.vector.tensor_add(
                    zsum[:, t * 256:(t + 1) * 256],
                    zn[:, t * 256:(t + 1) * 256],
                    zn[:, (t + 16) * 256:(t + 17) * 256],
                )
            # ---------------- phase C: f32r sim + exp row sums ----------------
            for j in range(4):
                for rt in range(4):
                    ps = pc.tile([128, 1024], f32, tag="sim")
                    for cc in range(2):
                        col0 = j * 1024 + cc * 512
                        for d in range(2):
                            nc.tensor.matmul(
                                ps[:, cc * 512:(cc + 1) * 512],
                                lhsT=zqT_bf[:, d * 512 + rt * 128: d * 512 + (rt + 1) * 128],
                                rhs=zT[:, d * 4096 + col0: d * 4096 + col0 + 512],
                                start=(d == 0), stop=(d == 1),
                            )
                    nc.scalar.activation(
                        exp_scr[:], ps[:], EXP,
                        accum_out=S_parts[:, rt * 4 + j: rt * 4 + j + 1],
                    )

            # ---------------- nominator (after C in issue order) ----------------
            with tc.tile_pool(name="psum_b", bufs=1, space="PSUM") as pb:
                wt_ps = pb.tile([128, 256], f32, tag="wt")
                for t in range(16):
                    nc.tensor.matmul(
                        wt_ps[0:C, :],
                        lhsT=dist_sb[:, t * C:(t + 1) * C],
                        rhs=zsum[:, t * 256:(t + 1) * 256],
                        start=(t == 0), stop=(t == 15),
                    )
                nc.vector.tensor_copy(out=wt_sb[0:C, :], in_=wt_ps[0:C, :])
                for d in range(2):
                    w_ps = pb.tile([128, 128], f32, tag="wq")
                    nc.tensor.transpose(
                        w_ps[:, 0:C],
                        wt_sb[0:C, d * 128:(d + 1) * 128],
                        ident[0:C, 0:C],
                    )
                    nc.vector.tensor_copy(out=w_sb[:, d * C:(d + 1) * C],
                                          in_=w_ps[:, 0:C])

                for rt in range(4):
                    q_ps = pb.tile([128, 128], f32, tag="wq")
                    for d in range(2):
                        nc.tensor.matmul(
                            q_ps[:, 0:C],
                            lhsT=zqT[:, d * 512 + rt * 128: d * 512 + (rt + 1) * 128],
                            rhs=w_sb[:, d * C:(d + 1) * C],
                            start=(d == 0), stop=(d == 1),
                        )
                    nc.vector.tensor_copy(out=q_sb[:], in_=q_ps[:, 0:C])
                    nc.vector.tensor_mul(junk[:, 0:C], q_sb[:],
                                         distq_sb[:, rt * C:(rt + 1) * C])
                    nc.vector.tensor_reduce(out=P_out[:, rt:rt + 1],
                                            in_=junk[:, 0:C], axis=AX, op=ADD)
                nc.vector.tensor_scalar(out=P_out[:], in0=P_out[:], scalar1=-1.0,
                                        scalar2=None, op0=ADD)

            for rt in range(4):
                nc.vector.tensor_reduce(
                    out=S_out[:, rt:rt + 1], in_=S_parts[:, rt * 4:rt * 4 + 4],
                    axis=AX, op=ADD,
                )
            nc.vector.tensor_scalar(out=S_out[:], in0=S_out[:],
                                    scalar1=-E_CONST, scalar2=None, op0=ADD)

        nc.sync.dma_start(out=out[:, 0:4], in_=S_out[:])
        nc.sync.dma_start(out=out[:, 4:8], in_=P_out[:])
        nc.sync.dma_start(out=out[:, 8:9], in_=warm[:])

    nc.finalize()
    return nc


def _get_program():
    global _PROG
    if _PROG is None:
        _PROG = _build_program()
    return _PROG


def kernel(z_i, z_j, z_n, dist_labels):
    from concourse.bass_utils import run_bass_kernel_spmd

    nc = _get_program()

    z_full = np.ascontiguousarray(
        np.concatenate([z_i, z_j], axis=0), dtype=np.float32
    )
    dist = np.ascontiguousarray(dist_labels, dtype=np.float32)

    in_maps = []
    for c in range(N_CORES):
        r0 = c * RPC
        in_maps.append({
            "z_full": z_full,
            "zq": np.ascontiguousarray(z_full[r0:r0 + RPC]),
            "dist": dist,
            "distq": np.ascontiguousarray(dist[r0 % B: r0 % B + RPC]),
        })

    res = run_bass_kernel_spmd(nc, in_maps, list(range(N_CORES))).results

    S = np.empty(N, np.float64)
    P = np.empty(N, np.float64)
    for c in range(N_CORES):
        o = res[c]["out"]
        S[c * RPC:(c + 1) * RPC] = o[:, 0:4].T.reshape(RPC).astype(np.float64)
        P[c * RPC:(c + 1) * RPC] = o[:, 4:8].T.reshape(RPC).astype(np.float64)

    return np.float32((P / S).sum() / N)



# revision 10
# speedup vs baseline: 1.0677x; 1.0677x over previous
"""NT-Xent loss kernel, v3.

vs v2: host-side rotation makes every core's q-block rows 0-511 so
zq/distq/zqT are views (no extra DMAs/transposes); normalization is
folded into the PE transposes via diag(rn) as the moving operand;
rn computed with integer rsqrt + Newton on GpSimd (no Ln -> single
exp table set, no table thrash); norms via one-pass
tensor_tensor_reduce; f32r matmuls for sim and W (1 cyc/row, full
fp32 data, no bf16 conversion copies); W built from rn-scaled dist
tiles; 8-chunk z DMA for finer pipelining; single merged output DMA.
"""

import numpy as np
from contextlib import ExitStack

N = 4096
D = 256
C = 100
B = 2048
N_CORES = 8
RPC = 512
NT = 32           # z row-tiles of 128
NCH = 8           # z DMA chunks (4 tiles each)
E_CONST = float(np.e)
MAGIC = 0x5F3759DF

_PROG = None


def _build_program():
    import concourse.bass as bass
    import concourse.tile as tile
    from concourse import bacc, mybir, masks

    f32 = mybir.dt.float32
    f32r = mybir.dt.float32r
    i32 = mybir.dt.int32
    MULT = mybir.AluOpType.mult
    ADD = mybir.AluOpType.add
    XOR = mybir.AluOpType.bitwise_xor
    SHR = mybir.AluOpType.arith_shift_right
    EXP = mybir.ActivationFunctionType.Exp
    SQUARE = mybir.ActivationFunctionType.Square
    AX = mybir.AxisListType.X

    nc = bacc.Bacc(
        "TRN2",
        target_bir_lowering=False,
        debug=False,
        enable_asserts=False,
        num_devices=N_CORES,
    )

    z = nc.dram_tensor("z", [N, D], f32r, kind="ExternalInput").ap()
    dist = nc.dram_tensor("dist", [B, C], f32r, kind="ExternalInput").ap()
    out = nc.dram_tensor("out", [128, 8], f32, kind="ExternalOutput").ap()

    with tile.TileContext(nc) as tc, ExitStack() as ctx:
        per = ctx.enter_context(tc.tile_pool(name="persist", bufs=1))

        ident = per.tile([128, 128], f32)
        masks.make_identity(nc, ident[:])
        ident_r = per.tile([128, 128], f32r)
        nc.vector.tensor_copy(out=ident_r[:], in_=ident[:])

        zn = per.tile([128, NT * 256], f32r)
        zT = per.tile([128, 2 * 4096], f32r)
        dist_sb = per.tile([128, 16 * C], f32r)
        norm2 = per.tile([128, NT], f32)
        rn = per.tile([128, NT], f32)
        hh = per.tile([128, NT], f32)
        wt_sb = per.tile([128, 256], f32)
        w_sb = per.tile([128, 2 * C], f32r)
        q_sb = per.tile([128, C], f32)
        junk = per.tile([128, C], f32)
        S_parts = per.tile([128, 16], f32)
        out_sb = per.tile([128, 8], f32)
        exp_scr = per.tile([128, 1024], f32)

        n2_i = norm2[:].bitcast(i32)
        rn_i = rn[:].bitcast(i32)
        hh_i = hh[:].bitcast(i32)

        # ---------------- DMAs: 8 z chunks (4 tiles each), then dist --------
        for ch in range(NCH):
            nc.sync.dma_start(
                out=zn[:, ch * 1024:(ch + 1) * 1024].rearrange(
                    "p (t j) -> p t j", j=256),
                in_=z[ch * 512:(ch + 1) * 512, :].rearrange(
                    "(t p) j -> p t j", p=128),
            )
        nc.sync.dma_start(
            out=dist_sb[:].rearrange("p (t c) -> p t c", c=C),
            in_=dist.rearrange("(t p) c -> p t c", p=128),
        )

        # ------- norms: fused Square+accum on ACT (early chunks), ---------
        # ------- two-pass mul+reduce on DVE (late chunks) ------------------
        sqp = ctx.enter_context(tc.tile_pool(name="sqp", bufs=2))

        def norms_chunk(ch):
            if ch < 3:
                sq = sqp.tile([128, 256], f32, tag="sqa")
                for t in range(ch * 4, ch * 4 + 4):
                    nc.scalar.activation(
                        sq[:], zn[:, t * 256:(t + 1) * 256].bitcast(f32),
                        SQUARE, accum_out=norm2[:, t:t + 1])
            else:
                sq = sqp.tile([128, 1024], f32, tag="sq")
                src_v = zn[:, ch * 1024:(ch + 1) * 1024].bitcast(f32)
                nc.vector.tensor_tensor(out=sq[:], in0=src_v, in1=src_v,
                                        op=MULT)
                nc.vector.tensor_reduce(
                    out=norm2[:, ch * 4:(ch + 1) * 4],
                    in_=sq[:].rearrange("p (t j) -> p t j", j=256),
                    axis=AX, op=ADD,
                )

        # ---------------- rsqrt batches on GpSimd (int Newton) --------------
        def rsqrt_batch(sl):
            # y0 = bitcast(MAGIC - (bitcast_i32(n2) >> 1)); 3 Newton steps
            nc.vector.tensor_scalar(out=hh_i[:, sl], in0=n2_i[:, sl],
                                    scalar1=1, scalar2=None, op0=SHR)
            nc.vector.tensor_scalar(out=rn_i[:, sl], in0=hh_i[:, sl],
                                    scalar1=-1, scalar2=None, op0=XOR)
            nc.vector.tensor_scalar(out=rn_i[:, sl], in0=rn_i[:, sl],
                                    scalar1=MAGIC + 1, scalar2=None, op0=ADD)
            for _ in range(3):
                nc.vector.tensor_tensor(out=hh[:, sl], in0=rn[:, sl],
                                        in1=rn[:, sl], op=MULT)
                nc.vector.tensor_tensor(out=hh[:, sl], in0=hh[:, sl],
                                        in1=norm2[:, sl], op=MULT)
                nc.vector.tensor_scalar(out=hh[:, sl], in0=hh[:, sl],
                                        scalar1=-0.5, scalar2=1.5,
                                        op0=MULT, op1=ADD)
                nc.vector.tensor_tensor(out=rn[:, sl], in0=rn[:, sl],
                                        in1=hh[:, sl], op=MULT)

        with tc.tile_pool(name="psum_tr", bufs=2, space="PSUM") as ptr, \
                tc.tile_pool(name="psum_c", bufs=3, space="PSUM") as pc:

            def transpose_chunk(ch):
                # 2 ptr tiles per chunk: same-d for the 4 z-tiles, so the
                # PSUM->SBUF copy lands contiguously in zT.
                for d in range(2):
                    pt = ptr.tile([128, 512], f32r, tag="tr")
                    for k in range(4):
                        t = ch * 4 + k
                        nc.tensor.transpose(
                            pt[:, k * 128:(k + 1) * 128],
                            zn[:, t * 256 + d * 128: t * 256 + d * 128 + 128],
                            ident_r[:],
                        )
                    nc.vector.tensor_copy(
                        out=zT[:, d * 4096 + ch * 512:(d * 4096 + (ch + 1) * 512)],
                        in_=pt[:])

            def scale_chunk(ch):
                # zn tile t *= rn_t in place (z -> z_hat)
                for t in range(ch * 4, ch * 4 + 4):
                    nc.vector.tensor_scalar(
                        out=zn[:, t * 256:(t + 1) * 256],
                        in0=zn[:, t * 256:(t + 1) * 256],
                        scalar1=rn[:, t:t + 1],
                        scalar2=None, op0=MULT)

            def sims_group(j):
                # sim rows 0-511 x cols [j*1024,(j+1)*1024), exp row-sums
                for rt in range(4):
                    ps = pc.tile([128, 1024], f32, tag="sim")
                    for cc in range(2):
                        col0 = j * 1024 + cc * 512
                        for d in range(2):
                            nc.tensor.matmul(
                                ps[:, cc * 512:(cc + 1) * 512],
                                lhsT=zT[:, d * 4096 + rt * 128:
                                        d * 4096 + (rt + 1) * 128],
                                rhs=zT[:, d * 4096 + col0:
                                       d * 4096 + col0 + 512],
                                start=(d == 0), stop=(d == 1),
                            )
                    nc.scalar.activation(
                        exp_scr[:], ps[:], EXP,
                        accum_out=S_parts[:, rt * 4 + j: rt * 4 + j + 1],
                    )

            # chunk pipeline: norms -> rsqrt -> scale -> transposes,
            # sims for col-group j after chunks 2j, 2j+1 are transposed.
            for ch in range(NCH):
                norms_chunk(ch)
                if ch < 2:
                    rsqrt_batch(slice(ch * 4, (ch + 1) * 4))
                    scale_chunk(ch)
                    transpose_chunk(ch)
                    if ch == 1:
                        sims_group(0)
                elif ch % 2 == 1:
                    rsqrt_batch(slice((ch - 1) * 4, (ch + 1) * 4))
                    scale_chunk(ch - 1)
                    scale_chunk(ch)
                    transpose_chunk(ch - 1)
                    transpose_chunk(ch)
                    sims_group(ch // 2)

        # ---------------- nominator: W = (rn*dist)^T z, P = zq_hat W -------
        with tc.tile_pool(name="psum_b", bufs=2, space="PSUM") as pb:
            wt_ps = pb.tile([128, 256], f32, tag="wt")
            for t in range(NT):
                nc.tensor.matmul(
                    wt_ps[0:C, :],
                    lhsT=dist_sb[:, (t % 16) * C:((t % 16) + 1) * C],
                    rhs=zn[:, t * 256:(t + 1) * 256],
                    start=(t == 0), stop=(t == NT - 1),
                )
            nc.vector.tensor_copy(out=wt_sb[0:C, :], in_=wt_ps[0:C, :])
            for d in range(2):
                w_ps = pb.tile([128, 128], f32, tag="wq")
                nc.tensor.transpose(
                    w_ps[:, 0:C],
                    wt_sb[0:C, d * 128:(d + 1) * 128],
                    ident[0:C, 0:C],
                )
                nc.vector.tensor_copy(out=w_sb[:, d * C:(d + 1) * C],
                                      in_=w_ps[:, 0:C])
            for rt in range(4):
                q_ps = pb.tile([128, 128], f32, tag="wq")
                for d in range(2):
                    nc.tensor.matmul(
                        q_ps[:, 0:C],
                        lhsT=zT[:, d * 4096 + rt * 128:
                                d * 4096 + (rt + 1) * 128],
                        rhs=w_sb[:, d * C:(d + 1) * C],
                        start=(d == 0), stop=(d == 1),
                    )
                nc.vector.tensor_copy(out=q_sb[:], in_=q_ps[:, 0:C])
                nc.vector.tensor_mul(junk[:], q_sb[:],
                                     dist_sb[:, rt * C:(rt + 1) * C].bitcast(f32))
                nc.vector.tensor_reduce(out=out_sb[:, 4 + rt:5 + rt],
                                        in_=junk[:], axis=AX, op=ADD)
            nc.vector.tensor_scalar(out=out_sb[:, 4:8], in0=out_sb[:, 4:8],
                                    scalar1=-1.0, scalar2=None, op0=ADD)

        for rt in range(4):
            nc.vector.tensor_reduce(
                out=out_sb[:, rt:rt + 1], in_=S_parts[:, rt * 4:rt * 4 + 4],
                axis=AX, op=ADD,
            )
        nc.vector.tensor_scalar(out=out_sb[:, 0:4], in0=out_sb[:, 0:4],
                                scalar1=-E_CONST, scalar2=None, op0=ADD)

        nc.sync.dma_start(out=out[:], in_=out_sb[:])

    nc.finalize()
    return nc


def _get_program():
    global _PROG
    if _PROG is None:
        _PROG = _build_program()
    return _PROG


def kernel(z_i, z_j, z_n, dist_labels):
    from concourse.bass_utils import run_bass_kernel_spmd

    nc = _get_program()

    z_full = np.ascontiguousarray(
        np.concatenate([z_i, z_j], axis=0), dtype=np.float32
    )
    dist = np.ascontiguousarray(dist_labels, dtype=np.float32)

    in_maps = []
    for c in range(N_CORES):
        r0 = c * RPC
        in_maps.append({
            "z": np.ascontiguousarray(np.roll(z_full, -r0, axis=0)),
            "dist": np.ascontiguousarray(np.roll(dist, -r0, axis=0)),
        })

    res = run_bass_kernel_spmd(nc, in_maps, list(range(N_CORES))).results

    S = np.empty(N, np.float64)
    P = np.empty(N, np.float64)
    for c in range(N_CORES):
        o = res[c]["out"]
        S[c * RPC:(c + 1) * RPC] = o[:, 0:4].T.reshape(RPC).astype(np.float64)
        P[c * RPC:(c + 1) * RPC] = o[:, 4:8].T.reshape(RPC).astype(np.float64)

    return np.float32((P / S).sum() / N)
